# revision 49
# baseline (speedup 1.0000x reference)
"""Causal self-attention (B=4, T=2048, C=1024, H=16) on 8 trn2 NeuronCores.

Sharding: core c -> (batch b = c//2, head-group g = c%2). Each core owns
heads 8g..8g+7 (feature dims 512g..512g+512) of its batch: it projects
q/k/v only for those 512 dims (no duplicated K/V work across cores),
runs attention for its 8 heads over the full causal sequence, and emits
a partial output projection; the host sums the two head-group partials
per batch and adds the output bias.

Per-core device pipeline (bf16 matmuls, fp32 PSUM accumulation):
  - Projections per head-pair hp (two heads share the 128-partition d):
    kT/qT in transposed [d, t] layout, v natural [t, d] with a ones
    column per head (AV matmul then also yields the softmax denom Z).
  - Attention per head-pair, query blocks J of 512 (natural order),
    key tiles of 128 with 128-granular causal trimming: S^T = K Q^T via
    row-packed tile_position matmuls (two heads concurrent), exp on the
    scalar engine (logits O(6), no max subtraction), diagonal tiles get
    a single 128x128 triangular multiplicative mask, AV accumulated
    over key tiles in PSUM.
  - Software pipelining: projection matmuls of head-pair hp+1 are
    interleaved into the attention kt-loop of hp so the tensor engine
    never waits on the scalar engine's exp stream.
  - Deferred normalization: 1/Z broadcast across partitions via K=1
    matmuls, applied to yT; output projection accumulates the 4 d-chunks
    and DMAs straight from PSUM.
"""

import numpy as np
import ml_dtypes
from contextlib import ExitStack

import concourse.bass as bass
import concourse.bacc as bacc
import concourse.mybir as mybir
import concourse.tile as tile
from concourse import bass_utils

B, T, C, H = 4, 2048, 1024, 16
HD = C // H            # 64
NCORES = 8
CG = C // 2            # 512 feature dims per core (8 heads)
NHP = CG // 128        # 4 head-pairs per core
NCH = C // 128         # 8 contraction chunks over C
NJ = T // 512          # 4 query blocks
SCALE = 1.0 / float(np.sqrt(HD))

bf16 = mybir.dt.bfloat16
f32 = mybir.dt.float32
AF = mybir.ActivationFunctionType

_compiled = {}
last_result = None  # BassKernelResults of the most recent run (for test harness)


def _build():
    nc = bacc.Bacc("TRN2", target_bir_lowering=False, debug=False,
                   num_devices=NCORES)

    # host pre-shuffles inputs to partition-major contiguous layouts so
    # every DMA moves 2-8KB per-partition lines; wq/wk are head-pair-major
    # and wv head-half-major so the first-needed slices are small DMAs
    xT_d = nc.dram_tensor("xTs", [128, NJ, NCH, 512], bf16,
                          kind="ExternalInput")
    wqT_d = nc.dram_tensor("wqs", [128, NHP, NCH, 128], bf16,
                           kind="ExternalInput")
    wkT_d = nc.dram_tensor("wks", [128, NHP, NCH, 128], bf16,
                           kind="ExternalInput")
    wvT_d = nc.dram_tensor("wvs", [128, 2, NCH, 256], bf16,
                           kind="ExternalInput")
    wpT_d = nc.dram_tensor("wps", [128, NHP, C], bf16, kind="ExternalInput")
    bq_d = nc.dram_tensor("bq2", [128, NHP], f32, kind="ExternalInput")
    mask_d = nc.dram_tensor("mask", [128, 256], bf16, kind="ExternalInput")
    out_d = nc.dram_tensor("out", [T, C], bf16, kind="ExternalOutput")

    xT_v = xT_d.ap()
    wq_v = wqT_d.ap()
    wk_v = wkT_d.ap()
    wv_v = wvT_d.ap()
    wp_v = wpT_d.ap()
    out_v = out_d.ap().rearrange("(a p) c -> p a c", p=128)

    with tile.TileContext(nc) as tc, ExitStack() as ctx:
        persist = ctx.enter_context(tc.tile_pool(name="persist", bufs=1))
        pp = ctx.enter_context(tc.tile_pool(name="pp", bufs=2, space="PSUM"))
        spool = ctx.enter_context(
            tc.tile_pool(name="spool", bufs=2, space="PSUM"))
        opool = ctx.enter_context(
            tc.tile_pool(name="opool", bufs=1, space="PSUM"))
        p2pool = ctx.enter_context(tc.tile_pool(name="p2pool", bufs=4))
        outp = ctx.enter_context(tc.tile_pool(name="outp", bufs=4))

        xT_sb = persist.tile([128, NCH, T], bf16)
        wq_sb = persist.tile([128, NHP, NCH, 128], bf16)
        wk_sb = persist.tile([128, NHP, NCH, 128], bf16)
        wv_sb = persist.tile([128, 2, NCH, 256], bf16)
        wp_sb = persist.tile([128, NHP, C], bf16)
        kT_sb = persist.tile([128, NHP, T], bf16)
        qT_sb = persist.tile([128, NHP, T], bf16)
        v_sb = persist.tile([128, 16, 8, HD + 1], bf16)
        yT_sb = persist.tile([128, NHP, T], bf16)
        # Z for (hp, J, head): partition 64*head + 32*(hp%2), slot 4*(hp//2)+J
        zst = persist.tile([128, 8, 512], f32)
        zr = persist.tile([128, 4, 512], f32)
        bq_sb = persist.tile([128, NHP], f32)
        mask_sb = persist.tile([128, 2, 128], bf16)
        ones_r = persist.tile([128, HD], f32)      # 1/Z broadcast matmul

        nc.vector.memset(ones_r[:], 1.0)
        nc.vector.memset(v_sb[:, :, :, HD:HD + 1], 1.0)
        nc.gpsimd.memset(zst[:], 1.0)

        # input DMAs: the cost model serializes DMAs on one global device at
        # ~360 GB/s, so order IS priority. Mask feeds the very first diag
        # tile; then the tb0 projection set; xT t-blocks ahead of the
        # weights for later head-pairs.
        nc.sync.dma_start(mask_sb[:], mask_d.ap())
        nc.sync.dma_start(bq_sb[:], bq_d.ap())
        nc.sync.dma_start(wk_sb[:, 0], wk_v[:, 0])
        nc.sync.dma_start(xT_sb[:, :, 0:512], xT_v[:, 0])
        nc.scalar.dma_start(wq_sb[:, 0], wq_v[:, 0])
        nc.scalar.dma_start(wv_sb[:, 0], wv_v[:, 0])
        nc.sync.dma_start(xT_sb[:, :, 512:1024], xT_v[:, 1])
        nc.sync.dma_start(xT_sb[:, :, 1024:1536], xT_v[:, 2])
        nc.scalar.dma_start(wk_sb[:, 1:4], wk_v[:, 1:4])
        nc.sync.dma_start(xT_sb[:, :, 1536:2048], xT_v[:, 3])
        nc.scalar.dma_start(wq_sb[:, 1:4], wq_v[:, 1:4])
        nc.scalar.dma_start(wp_sb[:], wp_v)
        nc.scalar.dma_start(wv_sb[:, 1], wv_v[:, 1])

        # ---------------- projection emitters (pipelined as work items) ----
        def proj_kq(w_sb, b_sb, dst_sb, hp, tb):
            """One 512-col t-block of the kT/qT projection for head-pair hp.

            b_sb None: bias skipped (k-bias is softmax-invariant: it adds a
            per-query constant q_i . bk to every logit of query i).
            """
            ps = pp.tile([128, 512], f32, tag="pp")
            ts = slice(512 * tb, 512 * tb + 512)
            for c in range(NCH):
                nc.tensor.matmul(
                    ps[:], w_sb[:, hp, c, :], xT_sb[:, c, ts],
                    start=(c == 0), stop=(c == NCH - 1))
            if b_sb is None:
                nc.vector.tensor_copy(dst_sb[:, hp, ts], ps[:])
            else:
                nc.vector.tensor_scalar_add(dst_sb[:, hp, ts], ps[:],
                                            b_sb[:, hp:hp + 1])

        def proj_v(half, r):
            """V rows [128r, 128r+128) for head-half `half` (v-bias folded
            into the host-side output bias: (y+Z*bv)/Z@Wp^T = y/Z@Wp^T+bv@Wp^T)."""
            ps = pp.tile([128, 512], f32, tag="pp")
            for c in range(NCH):
                nc.tensor.matmul(
                    ps[:, 0:256], xT_sb[:, c, 128 * r:128 * r + 128],
                    wv_sb[:, half, c, :], start=(c == 0), stop=(c == NCH - 1))
            nc.vector.tensor_copy(
                v_sb[:, r, 4 * half:4 * half + 4, 0:HD],
                ps[:, 0:256].rearrange("p (h e) -> p h e", e=HD))

        def norm_j(hp, J):
            """Normalize yT[hp, J] by 1/Z (reciprocal + K=1 broadcast mm)."""
            zslot = 4 * (hp // 2) + J
            pa = 32 * (hp % 2)        # Z row for head A
            pb = 64 + 32 * (hp % 2)   # Z row for head B
            qs = slice(512 * J, 512 * J + 512)
            nc.vector.reciprocal_approx_fast(zr[:, J, :], zst[:, zslot, :])
            bp1 = pp.tile([128, 512], f32, tag="pp")
            nc.tensor.matmul(bp1[0:64, :], ones_r[pa:pa + 1, :],
                             zr[pa:pa + 1, J, :], tile_position=(pa, 0))
            nc.tensor.matmul(bp1[64:128, :], ones_r[pb:pb + 1, :],
                             zr[pb:pb + 1, J, :], tile_position=(pb, 64))
            nc.vector.tensor_mul(yT_sb[:, hp, qs], yT_sb[:, hp, qs], bp1[:])

        def outproj_j(J):
            """Output projection for t-tiles of query block J (all heads)."""
            for tt in range(4 * J, 4 * J + 4):
                ot = outp.tile([128, 1024], bf16, tag="ot")
                for ch in range(2):
                    ps = pp.tile([128, 512], f32, tag="pp")
                    for d in range(NHP):
                        nc.tensor.matmul(
                            ps[:], yT_sb[:, d, 128 * tt:128 * tt + 128],
                            wp_sb[:, d, 512 * ch:512 * ch + 512],
                            start=(d == 0), stop=(d == NHP - 1))
                    if ch == 0:
                        nc.scalar.copy(ot[:, 0:512], ps[:])
                    else:
                        nc.vector.tensor_copy(ot[:, 512:1024], ps[:])
                nc.sync.dma_start(out_v[:, tt, :], ot[:])

        # ---------------- prologue (tb0 only; rest paced by DMA arrival) --
        proj_kq(wk_sb, None, kT_sb, 0, 0)
        proj_kq(wq_sb, bq_sb, qT_sb, 0, 0)
        proj_v(0, 0)
        proj_v(0, 1)

        # ---------------- attention, pipelined with next projections ------
        for hp in range(NHP):
            steps = sum(4 * J + 4 for J in range(NJ))  # 40
            sched = {}

            def put(s, item):
                sched.setdefault(s, []).append(item)

            if hp == 0:
                # own K0/Q0 tb1-3 paced to xT t-block DMA arrival (placing
                # them too early FIFO-blocks the exp stream behind the DMA);
                # V row r must precede AV(kt=r) of block J=r//4 (FIFO).
                put(2, ("kq", wk_sb, None, kT_sb, 0, 1))
                put(3, ("kq", wq_sb, bq_sb, qT_sb, 0, 1))
                put(10, ("kq", wk_sb, None, kT_sb, 0, 2))
                put(11, ("kq", wq_sb, bq_sb, qT_sb, 0, 2))
                put(21, ("kq", wk_sb, None, kT_sb, 0, 3))
                put(22, ("kq", wq_sb, bq_sb, qT_sb, 0, 3))
                for r, s in zip(range(2, 8), (1, 3, 5, 7, 9, 11)):
                    put(s, ("v", 0, r))
                for i, r in enumerate(range(8, 16)):
                    put(13 + i * 2, ("v", 0, r))
                kqsteps = [14, 16, 18, 20, 24, 26, 28, 30]
            else:
                kqsteps = [2, 6, 10, 14, 18, 22, 26, 30]
                if hp == 1:
                    for i, r in enumerate(range(16)):
                        put(1 + 2 * i, ("v", 1, r))
            if hp < NHP - 1:
                for tb in range(4):
                    put(kqsteps[tb], ("kq", wk_sb, None, kT_sb, hp + 1, tb))
                for tb in range(4):
                    put(kqsteps[4 + tb],
                        ("kq", wq_sb, bq_sb, qT_sb, hp + 1, tb))
            step = 0

            for J in range(NJ):
                qs = slice(512 * J, 512 * J + 512)
                oA = opool.tile([HD + 1, 512], f32, tag="oA")
                oB = opool.tile([HD + 1, 512], f32, tag="oB")
                pend = []
                for kt in range(4 * J + 4):
                    ks = slice(128 * kt, 128 * kt + 128)
                    i0 = 128 * (kt - 4 * J) if kt >= 4 * J else 0
                    s2 = spool.tile([128, 1024], f32, tag="s2")
                    nc.tensor.matmul(
                        s2[:, i0:512], kT_sb[0:64, hp, ks],
                        qT_sb[0:64, hp, 512 * J + i0:512 * J + 512],
                        tile_position=(0, 0))
                    nc.tensor.matmul(
                        s2[:, 512 + i0:1024], kT_sb[64:128, hp, ks],
                        qT_sb[64:128, hp, 512 * J + i0:512 * J + 512],
                        tile_position=(64, 0))
                    p2 = p2pool.tile([128, 1024], bf16, tag="p2")
                    if i0 == 0:
                        nc.scalar.activation(p2[:], s2[:], AF.Exp, scale=SCALE)
                    else:
                        s2v = s2[:].rearrange("p (h q) -> p h q", q=512)
                        p2v = p2[:].rearrange("p (h q) -> p h q", q=512)
                        nc.scalar.activation(p2v[:, :, i0:512],
                                             s2v[:, :, i0:512],
                                             AF.Exp, scale=SCALE)
                    if kt >= 4 * J:  # diagonal tile: 128x128 triangular mask
                        p2v = p2[:].rearrange("p (h q) -> p h q", q=512)
                        nc.vector.tensor_mul(p2v[:, :, i0:i0 + 128],
                                             p2v[:, :, i0:i0 + 128],
                                             mask_sb[:])
                    if len(pend) >= 2:
                        kp, pp2, j0 = pend.pop(0)
                        nc.tensor.matmul(oA[:, j0:512],
                                         v_sb[:, kp, 2 * hp, :],
                                         pp2[:, j0:512],
                                         start=(kp == 0), stop=False)
                        nc.tensor.matmul(oB[:, j0:512],
                                         v_sb[:, kp, 2 * hp + 1, :],
                                         pp2[:, 512 + j0:1024],
                                         start=(kp == 0), stop=False)
                    pend.append((kt, p2, i0))
                    for w in sched.get(step, ()):
                        if w[0] == "kq":
                            proj_kq(*w[1:])
                        else:
                            proj_v(w[1], w[2])
                    step += 1
                for kp, pp2, j0 in pend:
                    last = (kp == 4 * J + 3)
                    nc.tensor.matmul(oA[:, j0:512], v_sb[:, kp, 2 * hp, :],
                                     pp2[:, j0:512], start=(kp == 0),
                                     stop=last)
                    nc.tensor.matmul(oB[:, j0:512],
                                     v_sb[:, kp, 2 * hp + 1, :],
                                     pp2[:, 512 + j0:1024],
                                     start=(kp == 0), stop=last)
                # stash unnormalized y^T and Z (normalization deferred)
                nc.vector.tensor_copy(yT_sb[0:64, hp, qs], oA[0:HD, :])
                nc.vector.tensor_copy(yT_sb[64:128, hp, qs], oB[0:HD, :])
                za = 32 * (hp % 2)
                zslot = 4 * (hp // 2) + J
                nc.vector.tensor_copy(zst[za:za + 1, zslot, :],
                                      oA[HD:HD + 1, :])
                nc.vector.tensor_copy(zst[64 + za:65 + za, zslot, :],
                                      oB[HD:HD + 1, :])
                norm_j(hp, J)
                if hp == NHP - 1:
                    # output projection as filler in the Scalar-bound tail
                    outproj_j(J)

    nc.compile()
    return nc


def prep_in_maps(x, Wq, bq, Wk, bk, Wv, bv, Wp, bp):
    x = np.asarray(x, dtype=np.float32)
    Wq = np.asarray(Wq, dtype=np.float32)
    Wk = np.asarray(Wk, dtype=np.float32)
    Wv = np.asarray(Wv, dtype=np.float32)
    Wp = np.asarray(Wp, dtype=np.float32)
    bq = np.asarray(bq, dtype=np.float32)
    bk = np.asarray(bk, dtype=np.float32)
    bv = np.asarray(bv, dtype=np.float32)

    bf = ml_dtypes.bfloat16
    kk = np.arange(128)[:, None]
    jj = np.arange(128)[None, :]
    tri = (kk <= jj).astype(bf)
    mask2 = np.ascontiguousarray(np.concatenate([tri, tri], axis=1))

    def shuf_w(wT, nsl, width):
        # [C_in, nsl*width] -> [128 p, nsl, C_in//128, width]
        n = wT.shape[0] // 128
        return np.ascontiguousarray(
            wT.reshape(n, 128, nsl, width).transpose(1, 2, 0, 3)).astype(bf)

    # x[b].T is [C, T]; -> [128 p, 4 tb, 8 c, 512 t]
    xTs = [np.ascontiguousarray(
        x[b].T.reshape(NCH, 128, NJ, 512).transpose(1, 2, 0, 3)).astype(bf)
        for b in range(B)]
    gslices = [slice(0, CG), slice(CG, C)]
    in_maps = []
    for core in range(NCORES):
        b, g = core // 2, core % 2
        gs = gslices[g]
        in_maps.append({
            "xTs": xTs[b],
            "wqs": shuf_w(Wq[gs, :].T, NHP, 128),
            "wks": shuf_w(Wk[gs, :].T, NHP, 128),
            "wvs": shuf_w(Wv[gs, :].T, 2, 256),
            "wps": shuf_w(Wp[:, gs].T, 1, C).reshape(128, NHP, C),
            "bq2": np.ascontiguousarray(bq[gs].reshape(NHP, 128).T),
            "mask": mask2,
        })
    return in_maps


def kernel(x, Wq, bq, Wk, bk, Wv, bv, Wp, bp, **_ignored):
    global last_result
    bp = np.asarray(bp, dtype=np.float32)
    in_maps = prep_in_maps(x, Wq, bq, Wk, bk, Wv, bv, Wp, bp)

    if "nc" not in _compiled:
        _compiled["nc"] = _build()
    nc = _compiled["nc"]

    last_result = bass_utils.run_bass_kernel_spmd(
        nc, in_maps, core_ids=list(range(NCORES)))

    # v-bias folded here: y includes v without bias; (y/Z + bv) @ Wp^T + bp
    bp_eff = bp + np.asarray(bv, dtype=np.float32) @ np.asarray(
        Wp, dtype=np.float32).T
    out = np.empty((B, T, C), dtype=np.float32)
    for b in range(B):
        out[b] = np.asarray(last_result.results[2 * b]["out"],
                            dtype=np.float32)
        out[b] += np.asarray(last_result.results[2 * b + 1]["out"],
                             dtype=np.float32)
    out += bp_eff[None, None, :]
    return out
